# revision 28
# baseline (speedup 1.0000x reference)
"""AdvancedVectorMemory fused kernel for 8 Trainium2 NeuronCores.

Sharding: core c handles batch b = c//4 and heads 4*(c%4) .. 4*(c%4)+3
(data parallel over batch, tensor parallel over heads). Attention runs
flash-style per head pair with fused denominators (ones column in V).
Normalization is deferred past a bf16 AllGather that hands every core the
s-slice (512 rows) it owns for the output projection / gate epilogue.

Perf structure vs the f32 baseline:
 - exp batched in [128,1536] strips (3 psum banks) to amortize ACT
   instruction overhead; scalar engine drops just below the tensor
   engine's per-tile time.
 - AllGather payload is bf16 (raw retrieved + denominator rows), 2x
   smaller; output tensors are Shared scratchpad.
 - Wo/Wg1 weights are streamed to SBUF as bf16 and qs is preloaded
   during the attention phase (gpsimd-issued DMAs + DVE converts), so
   the epilogue never waits on HBM.
 - Wg1 @ q runs in the AllGather tail hole; Wg1 @ o is interleaved with
   the Wo accumulation.
"""
import sys
import numpy as np

for _p in ('/opt/trn_rl_repo', '/root/.axon_site/_ro/trn_rl_repo'):
    if _p not in sys.path:
        sys.path.insert(0, _p)

B, S, M = 2, 2048, 4096
DM, DK = 1024, 768
H, Dh = 16, 64
NC = 8
GS = 4           # group size (cores per batch)
SC_W = 512       # s-chunk width
N_SC = S // SC_W
N_MT = M // 128  # 32 m-tiles
SSL = S // GS    # per-core s-slice for the epilogue (512)

_PROG = None


def _build_program():
    from concourse import bacc, mybir, tile
    import concourse.bass as bass

    F32 = mybir.dt.float32
    F32R = mybir.dt.float32r
    BF16 = mybir.dt.bfloat16
    AF = mybir.ActivationFunctionType
    ALU = mybir.AluOpType

    nc = bacc.Bacc('TRN2', target_bir_lowering=False, debug=False, num_devices=NC)

    def din(name, shape, dt=F32R):
        return nc.dram_tensor(name, shape, dt, kind='ExternalInput').ap()

    qT = din('qT', [DM, S])
    mkT = din('mkT', [DK, M])
    mvT = din('mvT', [DK, M])
    wqT = din('wqT', [DM, 256])
    wkT = din('wkT', [DK, 256])
    wvT = din('wvT', [DK, 256])
    woT = din('woT', [DM, DM], F32)
    wg1T = din('wg1T', [2 * DM, DM], F32)
    wg2T = din('wg2T', [DM, 2])
    qsT = din('qsT', [DM, SSL], F32)
    bc0 = din('bc0', [2, 128])        # row0 = ones (gate broadcast)
    bqv = din('bqv', [2, 128], F32)
    bkv = din('bkv', [2, 128], F32)
    bo2v = din('bo2v', [8, 128], F32)
    bg1v = din('bg1v', [8, 128], F32)
    bg2v = din('bg2v', [2, 1], F32)
    vones = nc.dram_tensor('vones', [128, 8], BF16, kind='ExternalInput').ap()
    gidx = nc.dram_tensor('gidx', [8, 128], mybir.dt.int32, kind='ExternalInput').ap()
    didx = nc.dram_tensor('didx', [1, 128], mybir.dt.int32, kind='ExternalInput').ap()
    sel16 = din('sel16', [128, 1024])

    out_t = nc.dram_tensor('out_t', [DM, SSL], F32, kind='ExternalOutput').ap()

    groups = [list(range(GS)), list(range(GS, 2 * GS))]

    with tile.TileContext(nc) as tc:
        with tc.tile_pool(name='consts', bufs=1) as consts, \
             tc.tile_pool(name='pre', bufs=1) as pre, \
             tc.tile_pool(name='stage', bufs=3) as stage, \
             tc.tile_pool(name='dram', bufs=1, space='DRAM') as dram:

            # ---------------- small constants ----------------
            bq_sb = consts.tile([128, 2], F32, tag='bq_sb')
            bk_sb = consts.tile([128, 2], F32, tag='bk_sb')
            for p in range(2):
                nc.sync.dma_start(out=bq_sb[:, p:p + 1], in_=bqv[p:p + 1, :])
                nc.sync.dma_start(out=bk_sb[:, p:p + 1], in_=bkv[p:p + 1, :])
            gidx_sb = []
            for r in range(8):
                gt = consts.tile([128, 1], mybir.dt.int32, tag=f'gidx{r}',
                                 name=f'gidx{r}')
                nc.sync.dma_start(out=gt[:], in_=gidx[r:r + 1, :])
                gidx_sb.append(gt)
            didx8_sb = consts.tile([128, 1], mybir.dt.int32, tag='didx8')
            nc.sync.dma_start(out=didx8_sb[:], in_=didx[0:1, :])
            sel16_sb = consts.tile([128, 1024], F32R, tag='sel16_sb')
            nc.sync.dma_start(out=sel16_sb[:], in_=sel16[:])
            bc0_sb = consts.tile([2, 128], F32R, tag='bc0_sb')
            nc.sync.dma_start(out=bc0_sb[:], in_=bc0[:])
            bo2_sb = consts.tile([128, 8], F32, tag='bo2_sb')
            bg1_sb = consts.tile([128, 8], F32, tag='bg1_sb')
            for k in range(8):
                nc.gpsimd.dma_start(out=bo2_sb[:, k:k + 1], in_=bo2v[k:k + 1, :])
                nc.gpsimd.dma_start(out=bg1_sb[:, k:k + 1], in_=bg1v[k:k + 1, :])
            bg2_sb = consts.tile([2, 1], F32, tag='bg2_sb')
            nc.gpsimd.dma_start(out=bg2_sb[:], in_=bg2v[:])
            wg2_sb = consts.tile([128, 16], F32R, tag='wg2_sb')
            for k in range(8):
                nc.gpsimd.dma_start(out=wg2_sb[:, 2 * k:2 * (k + 1)],
                                    in_=wg2T[128 * k:128 * (k + 1), :])

            # epilogue tiles preloaded/converted during the attention phase
            wo_bf = pre.tile([128, 8 * DM], BF16, tag='wo_bf')
            wg1_bf = pre.tile([128, 16 * DM], BF16, tag='wg1_bf')
            qs_sb = pre.tile([128, 8 * SSL], F32, tag='qs_sb')
            qs_bf = pre.tile([128, 8 * SSL], BF16, tag='qs_bf')

            def preload_step(step):
                # 32 steps: wo chunks 0-7, wg1 chunks 8-23, qs slices 24-31.
                # DMAs issue from the sync queue AFTER each chunk's rt writes,
                # so they never starve the front-phase K/V stream.
                if step < 8:
                    kc = step
                    st = stage.tile([128, DM], F32, tag='st', name=f'st_wo{kc}')
                    nc.sync.dma_start(out=st[:], in_=woT[128 * kc:128 * (kc + 1), :])
                    nc.vector.tensor_copy(wo_bf[:, DM * kc:DM * (kc + 1)], st[:])
                elif step < 24:
                    kc = step - 8
                    st = stage.tile([128, DM], F32, tag='st', name=f'st_wg{kc}')
                    nc.sync.dma_start(out=st[:], in_=wg1T[128 * kc:128 * (kc + 1), :])
                    nc.vector.tensor_copy(wg1_bf[:, DM * kc:DM * (kc + 1)], st[:])
                else:
                    k = step - 24
                    nc.sync.dma_start(out=qs_sb[:, SSL * k:SSL * (k + 1)],
                                      in_=qsT[128 * k:128 * (k + 1), :])
                    nc.vector.tensor_copy(qs_bf[:, SSL * k:SSL * (k + 1)],
                                          qs_sb[:, SSL * k:SSL * (k + 1)])

            rt_in = dram.tile([1056, 512], BF16, tag='rt_in', name='rt_in')
            rt_out = dram.tile([8448, 512], BF16, tag='rt_out', name='rt_out')

            with tc.tile_pool(name='proj', bufs=1) as proj:
                # ---------------- phase A: projections ----------------
                qt_pair = [proj.tile([128, S], BF16, tag=f'qt_pair{p}',
                                     name=f'qt_pair{p}') for p in range(2)]
                kt_pair = [proj.tile([128, M], BF16, tag=f'kt_pair{p}',
                                     name=f'kt_pair{p}') for p in range(2)]
                v_sb = [proj.tile([128, 264], BF16, tag=f'v_sb{mt}',
                                  name=f'v_sb{mt}') for mt in range(N_MT)]

                with tc.tile_pool(name='qin', bufs=1) as qin, \
                     tc.tile_pool(name='psA', bufs=1, space='PSUM') as psA:
                    wq_sb = qin.tile([128, 2048], F32R, tag='wq_sb')
                    for k in range(8):
                        nc.sync.dma_start(out=wq_sb[:, 256 * k:256 * (k + 1)],
                                          in_=wqT[128 * k:128 * (k + 1), :])
                    qt_chunks = qin.tile([128, 8 * S], F32R, tag='qt_chunks')
                    for k in range(8):
                        nc.sync.dma_start(out=qt_chunks[:, S * k:S * (k + 1)],
                                          in_=qT[128 * k:128 * (k + 1), :])
                    pqs = [psA.tile([128, 512], F32, tag=f'pq{i}', name=f'pq{i}')
                           for i in range(8)]
                    for k in range(8):
                        for p in range(2):
                            for sc in range(N_SC):
                                nc.tensor.matmul(
                                    pqs[4 * p + sc][:],
                                    wq_sb[:, 256 * k + 128 * p:256 * k + 128 * (p + 1)],
                                    qt_chunks[:, S * k + SC_W * sc:S * k + SC_W * (sc + 1)],
                                    start=(k == 0), stop=(k == 7))
                    for p in range(2):
                        for sc in range(N_SC):
                            nc.vector.tensor_scalar_add(
                                qt_pair[p][:, SC_W * sc:SC_W * (sc + 1)],
                                pqs[4 * p + sc][:], bq_sb[:, p:p + 1])

                # ---------------- K/V projections (own psum, closes early) ----
                with tc.tile_pool(name='mkin', bufs=2) as mkin, \
                     tc.tile_pool(name='kvw', bufs=1) as kvw, \
                     tc.tile_pool(name='psK', bufs=1, space='PSUM') as psK, \
                     tc.tile_pool(name='psV', bufs=1, space='PSUM') as psV:
                    wk_sb = kvw.tile([128, 1536], F32R, tag='wk_sb')
                    wv_sb = kvw.tile([128, 1536], F32R, tag='wv_sb')
                    for k in range(6):
                        nc.sync.dma_start(out=wk_sb[:, 256 * k:256 * (k + 1)],
                                          in_=wkT[128 * k:128 * (k + 1), :])
                        nc.sync.dma_start(out=wv_sb[:, 256 * k:256 * (k + 1)],
                                          in_=wvT[128 * k:128 * (k + 1), :])
                    for mc in range(8):  # m blocks of 512
                        mkb = mkin.tile([128, 3072], F32R, tag='mkb')
                        mvb = mkin.tile([128, 3072], F32R, tag='mvb')
                        for k in range(6):
                            nc.sync.dma_start(
                                out=mkb[:, 512 * k:512 * (k + 1)],
                                in_=mkT[128 * k:128 * (k + 1), 512 * mc:512 * (mc + 1)])
                            nc.sync.dma_start(
                                out=mvb[:, 512 * k:512 * (k + 1)],
                                in_=mvT[128 * k:128 * (k + 1), 512 * mc:512 * (mc + 1)])
                        for p in range(2):
                            pk = psK.tile([128, 512], F32, tag='pk')
                            for k in range(6):
                                nc.tensor.matmul(
                                    pk[:],
                                    wk_sb[:, 256 * k + 128 * p:256 * k + 128 * (p + 1)],
                                    mkb[:, 512 * k:512 * (k + 1)],
                                    start=(k == 0), stop=(k == 5))
                            nc.vector.tensor_scalar_add(
                                kt_pair[p][:, 512 * mc:512 * (mc + 1)], pk[:],
                                bk_sb[:, p:p + 1])
                        for ml in range(4):
                            mt = 4 * mc + ml
                            pv = psV.tile([128, 256], F32, tag='pv')
                            for k in range(6):
                                nc.tensor.matmul(
                                    pv[:],
                                    mvb[:, 512 * k + 128 * ml:512 * k + 128 * (ml + 1)],
                                    wv_sb[:, 256 * k:256 * (k + 1)],
                                    start=(k == 0), stop=(k == 5))
                            vh = v_sb[mt].rearrange('p (h c) -> p h c', h=4)
                            nc.sync.dma_start(
                                out=vh[:, :, 64:66],
                                in_=vones[:].rearrange('p (h c) -> p h c', h=4))
                            nc.vector.tensor_copy(
                                vh[:, :, 0:64],
                                pv[:].rearrange('p (h d) -> p h d', h=4))

                # ---------------- attention ----------------
                with tc.tile_pool(name='attn', bufs=4) as apool, \
                     tc.tile_pool(name='rtst', bufs=4) as rtst, \
                     tc.tile_pool(name='psQK', bufs=2, space='PSUM') as psQK, \
                     tc.tile_pool(name='psAV', bufs=1, space='PSUM') as psAV:
                    for sc in range(N_SC):
                        for p in range(2):
                            accA = psAV.tile([66, 512], F32, tag='accA')
                            accB = psAV.tile([66, 512], F32, tag='accB')
                            acc = [accA, accB]
                            # 64 slices (mt, head); exp in strips of 3
                            j = 0
                            while j < 64:
                                gw = min(3, 64 - j)
                                tQ = psQK.tile([128, 1536], F32, tag='tQ')
                                for k in range(gw):
                                    mt, h = (j + k) // 2, (j + k) % 2
                                    nc.tensor.matmul(
                                        tQ[:, 512 * k:512 * (k + 1)],
                                        kt_pair[p][64 * h:64 * (h + 1),
                                                   128 * mt:128 * (mt + 1)],
                                        qt_pair[p][64 * h:64 * (h + 1),
                                                   SC_W * sc:SC_W * (sc + 1)],
                                        start=True, stop=True)
                                at = apool.tile([128, 1536], BF16, tag='at')
                                nc.scalar.activation(at[:, 0:512 * gw],
                                                     tQ[:, 0:512 * gw], AF.Exp)
                                for k in range(gw):
                                    mt, h = (j + k) // 2, (j + k) % 2
                                    nc.tensor.matmul(
                                        acc[h][:],
                                        v_sb[mt][:, 66 * (2 * p + h):66 * (2 * p + h) + 66],
                                        at[:, 512 * k:512 * (k + 1)],
                                        start=(mt == 0), stop=(mt == N_MT - 1))
                                j += gw
                            for h in range(2):
                                rt_t = rtst.tile([66, 512], BF16, tag='rt_t',
                                                 name=f'rt_t{p}{h}')
                                nc.vector.tensor_copy(rt_t[:], acc[h][0:66, :])
                                # pre-send the reciprocal of the denominator row
                                # (same partition offset for in/out — DVE ops
                                # are lane-wise)
                                rcp = rtst.tile([66, 512], F32, tag='rcp',
                                                name=f'rcp{p}{h}')
                                nc.vector.reciprocal(
                                    rcp[64:65, :], acc[h][64:65, :])
                                nc.vector.tensor_copy(rt_t[64:65, :],
                                                      rcp[64:65, :])
                                nc.sync.dma_start(
                                    out=rt_in[264 * sc + 132 * p + 66 * h:
                                              264 * sc + 132 * p + 66 * (h + 1), :],
                                    in_=rt_t[:])
                        nc.gpsimd.collective_compute(
                            'AllGather', ALU.bypass,
                            replica_groups=[list(range(NC))],
                            ins=[rt_in[264 * sc:264 * (sc + 1), :].opt()],
                            outs=[rt_out[2112 * sc:2112 * (sc + 1), :].opt()])
                        # stream epilogue weights in behind this chunk pair
                        for st_i in range(8 * sc, 8 * (sc + 1)):
                            preload_step(st_i)

            # ---------------- epilogue (own s-slice) ----------------
            with tc.tile_pool(name='ep', bufs=1) as ep, \
                 tc.tile_pool(name='ept', bufs=3) as ept:
                gq_sb = ep.tile([128, 8 * 512], F32, tag='gq_sb')
                # Wg1 @ q for both hidden halves — runs in the AllGather hole
                for half in range(2):
                    with tc.tile_pool(name=f'psGQ{half}', bufs=1,
                                      space='PSUM') as psGQ:
                        pgq = [psGQ.tile([128, 512], F32, tag=f'pgq{i}',
                                         name=f'pgq{half}_{i}') for i in range(4)]
                        for kc in range(8):
                            for i in range(4):
                                dt = 4 * half + i
                                nc.tensor.matmul(
                                    pgq[i][:],
                                    wg1_bf[:, DM * kc + 128 * dt:DM * kc + 128 * (dt + 1)],
                                    qs_bf[:, SSL * kc:SSL * (kc + 1)],
                                    start=(kc == 0), stop=(kc == 7))
                        for i in range(4):
                            dt = 4 * half + i
                            nc.vector.tensor_copy(
                                gq_sb[:, 512 * dt:512 * (dt + 1)], pgq[i][:])

                # gathered denominator-reciprocal rows (pre-inverted at send)
                dgt = ept.tile([128, 512], BF16, tag='dgt', name='dgt')
                nc.gpsimd.indirect_dma_start(
                    out=dgt[:], out_offset=None, in_=rt_out[:],
                    in_offset=bass.IndirectOffsetOnAxis(ap=didx8_sb[:], axis=0))
                rd = ep.tile([128, 512], F32R, tag='rd', name='rd')
                nc.vector.tensor_copy(rd[:], dgt[:])

                # gather + normalize retrieved chunks (bf16)
                kc_order = list(range(8))
                rtn = ep.tile([128, 8 * 512], BF16, tag='rtn')
                with tc.tile_pool(name='psN', bufs=2, space='PSUM') as psN:
                    for kc in kc_order:
                        raw = ept.tile([128, 512], BF16, tag='raw')
                        nc.gpsimd.indirect_dma_start(
                            out=raw[:], out_offset=None, in_=rt_out[:],
                            in_offset=bass.IndirectOffsetOnAxis(ap=gidx_sb[kc][:], axis=0))
                        bcp = psN.tile([128, 512], F32, tag='bcp')
                        nc.tensor.matmul(bcp[:], sel16_sb[:, 128 * kc:128 * (kc + 1)],
                                         rd[:], start=True, stop=True)
                        nc.vector.tensor_tensor(
                            rtn[:, 512 * kc:512 * (kc + 1)], raw[:], bcp[:], ALU.mult)

                # Wo projection interleaved with Wg1@o half 0
                oT = ep.tile([128, 8 * SSL], BF16, tag='oT')
                sl = ep.tile([128, 8 * 512], F32R, tag='sl')

                def silu_block(pg, dt):
                    hg = ept.tile([128, 512], F32, tag='hg', name=f'hg{dt}')
                    nc.vector.tensor_tensor(
                        hg[:], pg[:], gq_sb[:, 512 * dt:512 * (dt + 1)], ALU.add)
                    sg = ept.tile([128, 512], F32, tag='sg', name=f'sg{dt}')
                    nc.scalar.activation(sg[:], hg[:], AF.Sigmoid,
                                         bias=bg1_sb[:, dt:dt + 1])
                    gg = ept.tile([128, 512], F32, tag='gg', name=f'gg{dt}')
                    nc.vector.tensor_scalar_add(gg[:], hg[:], bg1_sb[:, dt:dt + 1])
                    nc.vector.tensor_tensor(
                        sl[:, 512 * dt:512 * (dt + 1)], gg[:], sg[:], ALU.mult)

                ctx_psG0 = tc.tile_pool(name='psG0', bufs=1, space='PSUM')
                psG0 = ctx_psG0.__enter__()
                ctx_psWo = tc.tile_pool(name='psWo', bufs=2, space='PSUM')
                psWo = ctx_psWo.__enter__()
                pgo0 = [psG0.tile([128, 512], F32, tag=f'pgo0_{i}',
                                  name=f'pgo0_{i}') for i in range(4)]
                def wg1o_step(pgo, half, dt, start, stop):
                    kc = 8 + dt
                    for i in range(4):
                        nc.tensor.matmul(
                            pgo[i][:],
                            wg1_bf[:, DM * kc + 512 * half + 128 * i:
                                   DM * kc + 512 * half + 128 * (i + 1)],
                            oT[:, SSL * dt:SSL * (dt + 1)],
                            start=start, stop=stop)

                for dt in range(8):
                    po = psWo.tile([128, 512], F32, tag='po')
                    for i, kc in enumerate(kc_order):
                        nc.tensor.matmul(
                            po[:], wo_bf[:, DM * kc + 128 * dt:DM * kc + 128 * (dt + 1)],
                            rtn[:, 512 * kc:512 * (kc + 1)],
                            start=(i == 0), stop=(i == 7))
                    nc.vector.tensor_scalar_add(
                        oT[:, SSL * dt:SSL * (dt + 1)], po[:], bo2_sb[:, dt:dt + 1])
                    # Wg1 @ o (half 0) lags one dt so PE never waits on the
                    # DVE bias-add that materializes oT
                    if dt >= 1:
                        wg1o_step(pgo0, 0, dt - 1, start=(dt == 1), stop=False)
                ctx_psWo.__exit__(None, None, None)
                wg1o_step(pgo0, 0, 7, start=False, stop=True)
                for i in range(4):
                    silu_block(pgo0[i], i)
                ctx_psG0.__exit__(None, None, None)

                with tc.tile_pool(name='psG1', bufs=1, space='PSUM') as psG1:
                    pgo1 = [psG1.tile([128, 512], F32, tag=f'pgo1_{i}',
                                      name=f'pgo1_{i}') for i in range(4)]
                    for dt in range(8):
                        wg1o_step(pgo1, 1, dt, start=(dt == 0), stop=(dt == 7))
                    for i in range(4):
                        silu_block(pgo1[i], 4 + i)

                # gate scalar: sigmoid(Wg2 @ sl + bg2), broadcast to 128 rows
                with tc.tile_pool(name='psT', bufs=1, space='PSUM') as psT:
                    pgt = psT.tile([2, 512], F32, tag='pgt')
                    for kc in range(8):
                        nc.tensor.matmul(pgt[:], wg2_sb[:, 2 * kc:2 * (kc + 1)],
                                         sl[:, 512 * kc:512 * (kc + 1)],
                                         start=(kc == 0), stop=(kc == 7))
                    gate = ep.tile([2, 512], F32R, tag='gate')
                    nc.scalar.activation(gate[:], pgt[:], AF.Sigmoid, bias=bg2_sb[:])
                    gb = psT.tile([128, 512], F32, tag='gb')
                    nc.tensor.matmul(gb[:], bc0_sb[:], gate[:], start=True, stop=True)
                    gbs = ep.tile([128, 512], F32, tag='gbs')
                    nc.vector.tensor_copy(gbs[:], gb[:])

                    # out = q + gate * o
                    for dt in range(8):
                        go = ept.tile([128, 512], F32, tag='go')
                        nc.vector.tensor_tensor(
                            go[:], gbs[:], oT[:, SSL * dt:SSL * (dt + 1)], ALU.mult)
                        fo = ept.tile([128, 512], F32, tag='fo')
                        nc.vector.tensor_tensor(
                            fo[:], go[:], qs_sb[:, SSL * dt:SSL * (dt + 1)], ALU.add)
                        nc.sync.dma_start(out=out_t[128 * dt:128 * (dt + 1), :],
                                          in_=fo[:])

    nc.compile()
    return nc


def _shard(inputs):
    import ml_dtypes
    _bf16 = ml_dtypes.bfloat16
    q = np.asarray(inputs['query'], np.float32)
    mk = np.asarray(inputs['memory_keys'], np.float32)
    mv = np.asarray(inputs['memory_values'], np.float32)
    Wq = np.asarray(inputs['Wq'], np.float32); bq = np.asarray(inputs['bq'], np.float32)
    Wk = np.asarray(inputs['Wk'], np.float32); bk = np.asarray(inputs['bk'], np.float32)
    Wv = np.asarray(inputs['Wv'], np.float32); bv = np.asarray(inputs['bv'], np.float32)
    Wo = np.asarray(inputs['Wo'], np.float32); bo = np.asarray(inputs['bo'], np.float32)
    Wg1 = np.asarray(inputs['Wg1'], np.float32); bg1 = np.asarray(inputs['bg1'], np.float32)
    Wg2 = np.asarray(inputs['Wg2'], np.float32); bg2 = np.asarray(inputs['bg2'], np.float32)

    scale = Dh ** -0.5
    bo2 = bo + Wo @ bv
    bc0 = np.zeros((2, 128), np.float32)
    bc0[0, :] = 1.0
    wg2T = np.zeros((DM, 2), np.float32)
    wg2T[:, 0] = Wg2[0]
    bg2v = np.zeros((2, 1), np.float32)
    bg2v[:, 0] = bg2[0]
    # sel16[row, 128*kc + j] = 1 where row = 4*r + 2*p + (j//64), kc = 2*r + p
    _sel16 = np.zeros((128, 1024), np.float32)
    for _kc in range(8):
        _r, _p = _kc // 2, _kc % 2
        for _j in range(128):
            _sel16[4 * _r + 2 * _p + _j // 64, 128 * _kc + _j] = 1.0

    qT_b = [np.ascontiguousarray(q[b].T) for b in range(B)]
    mkT_b = [np.ascontiguousarray(mk[b].T) for b in range(B)]
    mvT_b = [np.ascontiguousarray(mv[b].T) for b in range(B)]

    in_maps = []
    for c in range(NC):
        b, g = c // GS, c % GS
        hs = slice(64 * 4 * g, 64 * (4 * g + 4))  # rows of W for this core's 4 heads
        in_maps.append({
            'qT': qT_b[b],
            'mkT': mkT_b[b],
            'mvT': mvT_b[b],
            'wqT': np.ascontiguousarray((Wq[hs] * scale).T),
            'wkT': np.ascontiguousarray(Wk[hs].T),
            'wvT': np.ascontiguousarray(Wv[hs].T),
            'woT': np.ascontiguousarray(Wo.T),
            'wg1T': np.ascontiguousarray(Wg1.T),
            'wg2T': wg2T,
            'qsT': np.ascontiguousarray(q[b].T[:, SSL * g:SSL * (g + 1)]),
            'vones': np.ascontiguousarray(np.tile([1.0, 0.0], 4)[None, :].repeat(128, 0).astype(_bf16)),
            'bc0': bc0,
            'bqv': np.ascontiguousarray((bq[hs] * scale).reshape(2, 128)),
            'bkv': np.ascontiguousarray(bk[hs].reshape(2, 128)),
            'bo2v': np.ascontiguousarray(bo2.reshape(8, 128)),
            'bg1v': np.ascontiguousarray(bg1.reshape(8, 128)),
            'bg2v': bg2v,
            # row of (src r, pair p, head h, dim d) in rt_out:
            #   2112*g + 264*(4b + r) + 132*p + 66*h + d
            'gidx': np.asarray(
                [[2112 * g + 264 * (4 * b + kc // 2) + 132 * (kc % 2)
                  + 66 * (j // 64) + (j % 64)
                  for j in range(128)] for kc in range(8)], np.int32),
            'didx': np.asarray(
                [[2112 * g + 264 * (4 * b + j // 4) + 132 * ((j // 2) % 2)
                  + 66 * (j % 2) + 64 if j < 16
                  else 0 for j in range(128)]], np.int32),
            'sel16': _sel16,
        })
    return in_maps


def _run(inputs, trace=False):
    global _PROG
    from concourse.bass_utils import run_bass_kernel_spmd
    if _PROG is None:
        _PROG = _build_program()
    in_maps = _shard(inputs)
    res = run_bass_kernel_spmd(_PROG, in_maps, list(range(NC)), trace=trace)
    out = np.empty((B, S, DM), np.float32)
    for c in range(NC):
        b, g = c // GS, c % GS
        out[b, SSL * g:SSL * (g + 1), :] = res.results[c]['out_t'].T
    return out, res


def kernel(**inputs) -> np.ndarray:
    out, _ = _run(inputs, trace=False)
    return out


# revision 30
# speedup vs baseline: 1.1442x; 1.1442x over previous
"""AdvancedVectorMemory fused kernel for 8 Trainium2 NeuronCores.

Sharding: core c handles batch b = c//4 and heads 4*(c%4) .. 4*(c%4)+3
(data parallel over batch, tensor parallel over heads). Attention runs
flash-style per head pair with fused denominators (ones column in V).

Perf structure:
 - s-rotation: core (b, g) processes logical s-slices in the order
   g+1, g+2, g+3, g (mod 4), host-side permutation of q columns. Its
   own slice is computed LAST, so only 3 AllGathers are needed (the
   4th would carry data nobody else reads); each AG overlaps the next
   chunk pair's compute and the receive pipeline (gather + reciprocal
   of softmax denominators) runs during attention.
 - Wo / Wg1 input-channel blocks are host-permuted per core into
   gather-arrival order, so the epilogue consumes chunks uniformly.
 - exp batched in [128,1536] psum strips to amortize ACT overhead.
 - AllGather payload is bf16 raw retrieved + denominator rows.
 - Wo/Wg1 weights stream to SBUF as bf16 and qs preloads during the
   attention phase (sync-queue DMAs behind each chunk + DVE converts).
 - Wg1 @ q runs at the head of the tail; Wg1 @ o half 0 is interleaved
   with the Wo accumulation one dt behind.
"""
import sys
import numpy as np

for _p in ('/opt/trn_rl_repo', '/root/.axon_site/_ro/trn_rl_repo'):
    if _p not in sys.path:
        sys.path.insert(0, _p)

B, S, M = 2, 2048, 4096
DM, DK = 1024, 768
H, Dh = 16, 64
NC = 8
GS = 4           # group size (cores per batch)
SC_W = 512       # s-chunk width
N_SC = S // SC_W
N_MT = M // 128  # 32 m-tiles
SSL = S // GS    # per-core s-slice for the epilogue (512)

_PROG = None


def _build_program():
    from concourse import bacc, mybir, tile
    import concourse.bass as bass

    F32 = mybir.dt.float32
    F32R = mybir.dt.float32r
    BF16 = mybir.dt.bfloat16
    AF = mybir.ActivationFunctionType
    ALU = mybir.AluOpType

    nc = bacc.Bacc('TRN2', target_bir_lowering=False, debug=False, num_devices=NC)

    def din(name, shape, dt=F32R):
        return nc.dram_tensor(name, shape, dt, kind='ExternalInput').ap()

    qT = din('qT', [DM, S])
    mkT = din('mkT', [DK, M])
    mvT = din('mvT', [DK, M])
    wqT = din('wqT', [DM, 256])
    wkT = din('wkT', [DK, 256])
    wvT = din('wvT', [DK, 256])
    woT = din('woT', [DM, DM], F32)
    wg1T = din('wg1T', [2 * DM, DM], F32)
    wg2T = din('wg2T', [DM, 2])
    qsT = din('qsT', [DM, SSL], F32)
    bc0 = din('bc0', [2, 128])        # row0 = ones (gate broadcast)
    bqv = din('bqv', [2, 128], F32)
    bkv = din('bkv', [2, 128], F32)
    bo2v = din('bo2v', [8, 128], F32)
    bg1v = din('bg1v', [8, 128], F32)
    bg2v = din('bg2v', [2, 1], F32)
    vones = nc.dram_tensor('vones', [128, 8], BF16, kind='ExternalInput').ap()
    gidx = nc.dram_tensor('gidx', [8, 128], mybir.dt.int32, kind='ExternalInput').ap()
    didx = nc.dram_tensor('didx', [4, 128], mybir.dt.int32, kind='ExternalInput').ap()
    sel4 = din('sel4', [128, 256])

    out_t = nc.dram_tensor('out_t', [DM, SSL], F32, kind='ExternalOutput').ap()

    with tile.TileContext(nc) as tc:
        with tc.tile_pool(name='consts', bufs=1) as consts, \
             tc.tile_pool(name='pre', bufs=1) as pre, \
             tc.tile_pool(name='stage', bufs=3) as stage, \
             tc.tile_pool(name='dram', bufs=1, space='DRAM') as dram:

            # ---------------- small constants ----------------
            bq_sb = consts.tile([128, 2], F32, tag='bq_sb')
            bk_sb = consts.tile([128, 2], F32, tag='bk_sb')
            for p in range(2):
                nc.sync.dma_start(out=bq_sb[:, p:p + 1], in_=bqv[p:p + 1, :])
                nc.sync.dma_start(out=bk_sb[:, p:p + 1], in_=bkv[p:p + 1, :])
            gidx_sb = []
            for kc in range(8):
                gt = consts.tile([128, 1], mybir.dt.int32, tag=f'gidx{kc}',
                                 name=f'gidx{kc}')
                nc.sync.dma_start(out=gt[:], in_=gidx[kc:kc + 1, :])
                gidx_sb.append(gt)
            didx_sb = []
            for ci in range(4):
                dt_ = consts.tile([128, 1], mybir.dt.int32, tag=f'didx{ci}',
                                  name=f'didx{ci}')
                nc.sync.dma_start(out=dt_[:], in_=didx[ci:ci + 1, :])
                didx_sb.append(dt_)
            sel4_sb = consts.tile([128, 256], F32R, tag='sel4_sb')
            nc.sync.dma_start(out=sel4_sb[:], in_=sel4[:])
            bc0_sb = consts.tile([2, 128], F32R, tag='bc0_sb')
            nc.sync.dma_start(out=bc0_sb[:], in_=bc0[:])
            bo2_sb = consts.tile([128, 8], F32, tag='bo2_sb')
            bg1_sb = consts.tile([128, 8], F32, tag='bg1_sb')
            for k in range(8):
                nc.gpsimd.dma_start(out=bo2_sb[:, k:k + 1], in_=bo2v[k:k + 1, :])
                nc.gpsimd.dma_start(out=bg1_sb[:, k:k + 1], in_=bg1v[k:k + 1, :])
            bg2_sb = consts.tile([2, 1], F32, tag='bg2_sb')
            nc.gpsimd.dma_start(out=bg2_sb[:], in_=bg2v[:])
            wg2_sb = consts.tile([128, 16], F32R, tag='wg2_sb')
            for k in range(8):
                nc.gpsimd.dma_start(out=wg2_sb[:, 2 * k:2 * (k + 1)],
                                    in_=wg2T[128 * k:128 * (k + 1), :])

            # epilogue tiles preloaded/converted during the attention phase
            wo_bf = pre.tile([128, 8 * DM], BF16, tag='wo_bf')
            wg1_bf = pre.tile([128, 16 * DM], BF16, tag='wg1_bf')
            qs_sb = pre.tile([128, 8 * SSL], F32, tag='qs_sb')
            qs_bf = pre.tile([128, 8 * SSL], BF16, tag='qs_bf')
            # gathered raw retrieved chunks + denominator reciprocals
            rawk = pre.tile([128, 8 * 512], BF16, tag='rawk')
            rdr = pre.tile([128, 4 * 512], F32R, tag='rdr')

            def preload_step(step):
                # 32 steps: wo chunks 0-7, wg1 chunks 8-23, qs slices 24-31.
                # DMAs issue from the sync queue AFTER each chunk's rt writes,
                # so they never starve the front-phase K/V stream.
                if step < 8:
                    kc = step
                    st = stage.tile([128, DM], F32, tag='st', name=f'st_wo{kc}')
                    nc.sync.dma_start(out=st[:], in_=woT[128 * kc:128 * (kc + 1), :])
                    nc.vector.tensor_copy(wo_bf[:, DM * kc:DM * (kc + 1)], st[:])
                elif step < 24:
                    kc = step - 8
                    st = stage.tile([128, DM], F32, tag='st', name=f'st_wg{kc}')
                    nc.sync.dma_start(out=st[:], in_=wg1T[128 * kc:128 * (kc + 1), :])
                    nc.vector.tensor_copy(wg1_bf[:, DM * kc:DM * (kc + 1)], st[:])
                else:
                    k = step - 24
                    nc.sync.dma_start(out=qs_sb[:, SSL * k:SSL * (k + 1)],
                                      in_=qsT[128 * k:128 * (k + 1), :])
                    nc.vector.tensor_copy(qs_bf[:, SSL * k:SSL * (k + 1)],
                                          qs_sb[:, SSL * k:SSL * (k + 1)])

            rt_in = dram.tile([1056, 512], BF16, tag='rt_in', name='rt_in')
            rt_og = [dram.tile([2112, 512], BF16, tag=f'rt_og{i}',
                               name=f'rt_og{i}') for i in range(3)]

            def receive(ci, dpool):
                # gather arrival chunk ci (both pairs) + its denominator rows,
                # invert the denominators. ci<3 reads the AG output; ci=3 reads
                # this core's own rt_in rows (its own slice, computed last).
                src = rt_og[ci] if ci < 3 else rt_in
                for p in range(2):
                    kc = 2 * ci + p
                    nc.gpsimd.indirect_dma_start(
                        out=rawk[:, 512 * kc:512 * (kc + 1)], out_offset=None,
                        in_=src[:],
                        in_offset=bass.IndirectOffsetOnAxis(ap=gidx_sb[kc][:], axis=0))
                dgt = dpool.tile([128, 512], BF16, tag='dgt', name=f'dgt{ci}')
                nc.gpsimd.indirect_dma_start(
                    out=dgt[:], out_offset=None, in_=src[:],
                    in_offset=bass.IndirectOffsetOnAxis(ap=didx_sb[ci][:], axis=0))
                rdf = dpool.tile([128, 512], F32, tag='rdf', name=f'rdf{ci}')
                nc.vector.reciprocal(rdf[:], dgt[:])
                nc.vector.tensor_copy(rdr[:, 512 * ci:512 * (ci + 1)], rdf[:])

            with tc.tile_pool(name='proj', bufs=1) as proj:
                # ---------------- phase A: projections ----------------
                qt_pair = [proj.tile([128, S], BF16, tag=f'qt_pair{p}',
                                     name=f'qt_pair{p}') for p in range(2)]
                kt_pair = [proj.tile([128, M], BF16, tag=f'kt_pair{p}',
                                     name=f'kt_pair{p}') for p in range(2)]
                v_sb = [proj.tile([128, 264], BF16, tag=f'v_sb{mt}',
                                  name=f'v_sb{mt}') for mt in range(N_MT)]

                with tc.tile_pool(name='qw', bufs=1) as qw, \
                     tc.tile_pool(name='qin', bufs=2) as qin, \
                     tc.tile_pool(name='psA', bufs=1, space='PSUM') as psA:
                    wq_sb = qw.tile([128, 2048], F32R, tag='wq_sb')
                    for k in range(8):
                        nc.sync.dma_start(out=wq_sb[:, 256 * k:256 * (k + 1)],
                                          in_=wqT[128 * k:128 * (k + 1), :])
                    pqs = [psA.tile([128, 512], F32, tag=f'pq{i}', name=f'pq{i}')
                           for i in range(8)]
                    for kg in range(4):  # q k-chunks stream in pairs
                        qt_ch = qin.tile([128, 2 * S], F32R, tag='qt_ch')
                        for kk in range(2):
                            k = 2 * kg + kk
                            nc.sync.dma_start(out=qt_ch[:, S * kk:S * (kk + 1)],
                                              in_=qT[128 * k:128 * (k + 1), :])
                        for kk in range(2):
                            k = 2 * kg + kk
                            for p in range(2):
                                for sc in range(N_SC):
                                    nc.tensor.matmul(
                                        pqs[4 * p + sc][:],
                                        wq_sb[:, 256 * k + 128 * p:256 * k + 128 * (p + 1)],
                                        qt_ch[:, S * kk + SC_W * sc:S * kk + SC_W * (sc + 1)],
                                        start=(k == 0), stop=(k == 7))
                    for p in range(2):
                        for sc in range(N_SC):
                            nc.vector.tensor_scalar_add(
                                qt_pair[p][:, SC_W * sc:SC_W * (sc + 1)],
                                pqs[4 * p + sc][:], bq_sb[:, p:p + 1])

                # ---------------- K/V projections (own psum, closes early) ----
                with tc.tile_pool(name='mkin', bufs=2) as mkin, \
                     tc.tile_pool(name='kvw', bufs=1) as kvw, \
                     tc.tile_pool(name='psK', bufs=1, space='PSUM') as psK, \
                     tc.tile_pool(name='psV', bufs=1, space='PSUM') as psV:
                    wk_sb = kvw.tile([128, 1536], F32R, tag='wk_sb')
                    wv_sb = kvw.tile([128, 1536], F32R, tag='wv_sb')
                    for k in range(6):
                        nc.sync.dma_start(out=wk_sb[:, 256 * k:256 * (k + 1)],
                                          in_=wkT[128 * k:128 * (k + 1), :])
                        nc.sync.dma_start(out=wv_sb[:, 256 * k:256 * (k + 1)],
                                          in_=wvT[128 * k:128 * (k + 1), :])
                    for mc in range(8):  # m blocks of 512
                        mkb = mkin.tile([128, 3072], F32R, tag='mkb')
                        mvb = mkin.tile([128, 3072], F32R, tag='mvb')
                        for k in range(6):
                            nc.sync.dma_start(
                                out=mkb[:, 512 * k:512 * (k + 1)],
                                in_=mkT[128 * k:128 * (k + 1), 512 * mc:512 * (mc + 1)])
                            nc.sync.dma_start(
                                out=mvb[:, 512 * k:512 * (k + 1)],
                                in_=mvT[128 * k:128 * (k + 1), 512 * mc:512 * (mc + 1)])
                        for p in range(2):
                            pk = psK.tile([128, 512], F32, tag='pk')
                            for k in range(6):
                                nc.tensor.matmul(
                                    pk[:],
                                    wk_sb[:, 256 * k + 128 * p:256 * k + 128 * (p + 1)],
                                    mkb[:, 512 * k:512 * (k + 1)],
                                    start=(k == 0), stop=(k == 5))
                            nc.vector.tensor_scalar_add(
                                kt_pair[p][:, 512 * mc:512 * (mc + 1)], pk[:],
                                bk_sb[:, p:p + 1])
                        for ml in range(4):
                            mt = 4 * mc + ml
                            pv = psV.tile([128, 256], F32, tag='pv')
                            for k in range(6):
                                nc.tensor.matmul(
                                    pv[:],
                                    mvb[:, 512 * k + 128 * ml:512 * k + 128 * (ml + 1)],
                                    wv_sb[:, 256 * k:256 * (k + 1)],
                                    start=(k == 0), stop=(k == 5))
                            vh = v_sb[mt].rearrange('p (h c) -> p h c', h=4)
                            nc.sync.dma_start(
                                out=vh[:, :, 64:66],
                                in_=vones[:].rearrange('p (h c) -> p h c', h=4))
                            nc.vector.tensor_copy(
                                vh[:, :, 0:64],
                                pv[:].rearrange('p (h d) -> p h d', h=4))

                # ---------------- attention ----------------
                with tc.tile_pool(name='attn', bufs=4) as apool, \
                     tc.tile_pool(name='rtst', bufs=4) as rtst, \
                     tc.tile_pool(name='dge', bufs=2) as dge, \
                     tc.tile_pool(name='psQK', bufs=2, space='PSUM') as psQK, \
                     tc.tile_pool(name='psAV', bufs=1, space='PSUM') as psAV:
                    for sc in range(N_SC):
                        for p in range(2):
                            accA = psAV.tile([66, 512], F32, tag='accA')
                            accB = psAV.tile([66, 512], F32, tag='accB')
                            acc = [accA, accB]
                            # 64 slices (mt, head); exp in strips of 3
                            j = 0
                            while j < 64:
                                gw = min(3, 64 - j)
                                tQ = psQK.tile([128, 1536], F32, tag='tQ')
                                for k in range(gw):
                                    mt, h = (j + k) // 2, (j + k) % 2
                                    nc.tensor.matmul(
                                        tQ[:, 512 * k:512 * (k + 1)],
                                        kt_pair[p][64 * h:64 * (h + 1),
                                                   128 * mt:128 * (mt + 1)],
                                        qt_pair[p][64 * h:64 * (h + 1),
                                                   SC_W * sc:SC_W * (sc + 1)],
                                        start=True, stop=True)
                                at = apool.tile([128, 1536], BF16, tag='at')
                                nc.scalar.activation(at[:, 0:512 * gw],
                                                     tQ[:, 0:512 * gw], AF.Exp)
                                for k in range(gw):
                                    mt, h = (j + k) // 2, (j + k) % 2
                                    nc.tensor.matmul(
                                        acc[h][:],
                                        v_sb[mt][:, 66 * (2 * p + h):66 * (2 * p + h) + 66],
                                        at[:, 512 * k:512 * (k + 1)],
                                        start=(mt == 0), stop=(mt == N_MT - 1))
                                j += gw
                            for h in range(2):
                                rt_t = rtst.tile([66, 512], BF16, tag='rt_t',
                                                 name=f'rt_t{p}{h}')
                                nc.vector.tensor_copy(rt_t[:], acc[h][0:66, :])
                                nc.sync.dma_start(
                                    out=rt_in[264 * sc + 132 * p + 66 * h:
                                              264 * sc + 132 * p + 66 * (h + 1), :],
                                    in_=rt_t[:])
                        if sc < 3:
                            nc.gpsimd.collective_compute(
                                'AllGather', ALU.bypass,
                                replica_groups=[list(range(NC))],
                                ins=[rt_in[264 * sc:264 * (sc + 1), :].opt()],
                                outs=[rt_og[sc][:].opt()])
                        # stream epilogue weights in behind this chunk pair
                        for st_i in range(8 * sc, 8 * (sc + 1)):
                            preload_step(st_i)
                        # receive pipeline for the AG issued one pair ago
                        if sc >= 1:
                            receive(sc - 1, dge)
                    receive(2, dge)

            # ---------------- epilogue (own s-slice) ----------------
            with tc.tile_pool(name='ep', bufs=1) as ep, \
                 tc.tile_pool(name='ept', bufs=3) as ept:
                gq_sb = ep.tile([128, 8 * 512], F32, tag='gq_sb')
                # Wg1 @ q for both hidden halves — no AG dependency
                for half in range(2):
                    with tc.tile_pool(name=f'psGQ{half}', bufs=1,
                                      space='PSUM') as psGQ:
                        pgq = [psGQ.tile([128, 512], F32, tag=f'pgq{i}',
                                         name=f'pgq{half}_{i}') for i in range(4)]
                        for kc in range(8):
                            for i in range(4):
                                dt = 4 * half + i
                                nc.tensor.matmul(
                                    pgq[i][:],
                                    wg1_bf[:, DM * kc + 128 * dt:DM * kc + 128 * (dt + 1)],
                                    qs_bf[:, SSL * kc:SSL * (kc + 1)],
                                    start=(kc == 0), stop=(kc == 7))
                        for i in range(4):
                            dt = 4 * half + i
                            nc.vector.tensor_copy(
                                gq_sb[:, 512 * dt:512 * (dt + 1)], pgq[i][:])

                # own (last) chunk gather + denominators
                receive(3, ept)

                # normalize gathered chunks (bf16)
                rtn = ep.tile([128, 8 * 512], BF16, tag='rtn')
                with tc.tile_pool(name='psN', bufs=2, space='PSUM') as psN:
                    for kc in range(8):
                        ci, p = kc // 2, kc % 2
                        bcp = psN.tile([128, 512], F32, tag='bcp')
                        nc.tensor.matmul(bcp[:], sel4_sb[:, 128 * p:128 * (p + 1)],
                                         rdr[:, 512 * ci:512 * (ci + 1)],
                                         start=True, stop=True)
                        nc.vector.tensor_tensor(
                            rtn[:, 512 * kc:512 * (kc + 1)],
                            rawk[:, 512 * kc:512 * (kc + 1)], bcp[:], ALU.mult)

                # Wo projection interleaved with Wg1@o half 0
                oT = ep.tile([128, 8 * SSL], BF16, tag='oT')
                sl = ep.tile([128, 8 * 512], F32R, tag='sl')

                def silu_block(pg, dt):
                    hg = ept.tile([128, 512], F32, tag='hg', name=f'hg{dt}')
                    nc.vector.tensor_tensor(
                        hg[:], pg[:], gq_sb[:, 512 * dt:512 * (dt + 1)], ALU.add)
                    sg = ept.tile([128, 512], F32, tag='sg', name=f'sg{dt}')
                    nc.scalar.activation(sg[:], hg[:], AF.Sigmoid,
                                         bias=bg1_sb[:, dt:dt + 1])
                    gg = ept.tile([128, 512], F32, tag='gg', name=f'gg{dt}')
                    nc.vector.tensor_scalar_add(gg[:], hg[:], bg1_sb[:, dt:dt + 1])
                    nc.vector.tensor_tensor(
                        sl[:, 512 * dt:512 * (dt + 1)], gg[:], sg[:], ALU.mult)

                def wg1o_step(pgo, half, dt, start, stop):
                    kc = 8 + dt
                    for i in range(4):
                        nc.tensor.matmul(
                            pgo[i][:],
                            wg1_bf[:, DM * kc + 512 * half + 128 * i:
                                   DM * kc + 512 * half + 128 * (i + 1)],
                            oT[:, SSL * dt:SSL * (dt + 1)],
                            start=start, stop=stop)

                ctx_psG0 = tc.tile_pool(name='psG0', bufs=1, space='PSUM')
                psG0 = ctx_psG0.__enter__()
                pgo0 = [psG0.tile([128, 512], F32, tag=f'pgo0_{i}',
                                  name=f'pgo0_{i}') for i in range(4)]
                ctx_psWo = tc.tile_pool(name='psWo', bufs=2, space='PSUM')
                psWo = ctx_psWo.__enter__()
                for dt in range(8):
                    po = psWo.tile([128, 512], F32, tag='po')
                    for kc in range(8):
                        nc.tensor.matmul(
                            po[:], wo_bf[:, DM * kc + 128 * dt:DM * kc + 128 * (dt + 1)],
                            rtn[:, 512 * kc:512 * (kc + 1)],
                            start=(kc == 0), stop=(kc == 7))
                    nc.vector.tensor_scalar_add(
                        oT[:, SSL * dt:SSL * (dt + 1)], po[:], bo2_sb[:, dt:dt + 1])
                    # Wg1 @ o (half 0) lags one dt so PE never waits on the
                    # DVE bias-add that materializes oT
                    if dt >= 1:
                        wg1o_step(pgo0, 0, dt - 1, start=(dt == 1), stop=False)
                ctx_psWo.__exit__(None, None, None)
                wg1o_step(pgo0, 0, 7, start=False, stop=True)
                for i in range(4):
                    silu_block(pgo0[i], i)
                ctx_psG0.__exit__(None, None, None)

                with tc.tile_pool(name='psG1', bufs=1, space='PSUM') as psG1:
                    pgo1 = [psG1.tile([128, 512], F32, tag=f'pgo1_{i}',
                                      name=f'pgo1_{i}') for i in range(4)]
                    for dt in range(8):
                        wg1o_step(pgo1, 1, dt, start=(dt == 0), stop=(dt == 7))
                    for i in range(4):
                        silu_block(pgo1[i], 4 + i)

                # gate scalar: sigmoid(Wg2 @ sl + bg2), broadcast to 128 rows
                with tc.tile_pool(name='psT', bufs=1, space='PSUM') as psT:
                    pgt = psT.tile([2, 512], F32, tag='pgt')
                    for kc in range(8):
                        nc.tensor.matmul(pgt[:], wg2_sb[:, 2 * kc:2 * (kc + 1)],
                                         sl[:, 512 * kc:512 * (kc + 1)],
                                         start=(kc == 0), stop=(kc == 7))
                    gate = ep.tile([2, 512], F32R, tag='gate')
                    nc.scalar.activation(gate[:], pgt[:], AF.Sigmoid, bias=bg2_sb[:])
                    gb = psT.tile([128, 512], F32, tag='gb')
                    nc.tensor.matmul(gb[:], bc0_sb[:], gate[:], start=True, stop=True)
                    gbs = ep.tile([128, 512], F32, tag='gbs')
                    nc.vector.tensor_copy(gbs[:], gb[:])

                    # out = q + gate * o
                    for dt in range(8):
                        go = ept.tile([128, 512], F32, tag='go')
                        nc.vector.tensor_tensor(
                            go[:], gbs[:], oT[:, SSL * dt:SSL * (dt + 1)], ALU.mult)
                        fo = ept.tile([128, 512], F32, tag='fo')
                        nc.vector.tensor_tensor(
                            fo[:], go[:], qs_sb[:, SSL * dt:SSL * (dt + 1)], ALU.add)
                        nc.sync.dma_start(out=out_t[128 * dt:128 * (dt + 1), :],
                                          in_=fo[:])

    nc.compile()
    return nc


def _shard(inputs):
    import ml_dtypes
    _bf16 = ml_dtypes.bfloat16
    q = np.asarray(inputs['query'], np.float32)
    mk = np.asarray(inputs['memory_keys'], np.float32)
    mv = np.asarray(inputs['memory_values'], np.float32)
    Wq = np.asarray(inputs['Wq'], np.float32); bq = np.asarray(inputs['bq'], np.float32)
    Wk = np.asarray(inputs['Wk'], np.float32); bk = np.asarray(inputs['bk'], np.float32)
    Wv = np.asarray(inputs['Wv'], np.float32); bv = np.asarray(inputs['bv'], np.float32)
    Wo = np.asarray(inputs['Wo'], np.float32); bo = np.asarray(inputs['bo'], np.float32)
    Wg1 = np.asarray(inputs['Wg1'], np.float32); bg1 = np.asarray(inputs['bg1'], np.float32)
    Wg2 = np.asarray(inputs['Wg2'], np.float32); bg2 = np.asarray(inputs['bg2'], np.float32)

    scale = Dh ** -0.5
    bo2 = bo + Wo @ bv
    bc0 = np.zeros((2, 128), np.float32)
    bc0[0, :] = 1.0
    wg2T = np.zeros((DM, 2), np.float32)
    wg2T[:, 0] = Wg2[0]
    bg2v = np.zeros((2, 1), np.float32)
    bg2v[:, 0] = bg2[0]
    # sel4[2p + j//64, 128p + j] = 1 — picks denominator-recip row 2p+h
    _sel4 = np.zeros((128, 256), np.float32)
    for _p in range(2):
        for _j in range(128):
            _sel4[2 * _p + _j // 64, 128 * _p + _j] = 1.0

    qT_b = [np.ascontiguousarray(q[b].T) for b in range(B)]
    mkT_b = [np.ascontiguousarray(mk[b].T) for b in range(B)]
    mvT_b = [np.ascontiguousarray(mv[b].T) for b in range(B)]
    WoT = np.ascontiguousarray(Wo.T)     # [1024 in, 1024 out]
    Wg1T = np.ascontiguousarray(Wg1.T)   # [2048 in, 1024 out]

    in_maps = []
    for c in range(NC):
        b, g = c // GS, c % GS
        hs = slice(64 * 4 * g, 64 * (4 * g + 4))  # rows of W for this core's 4 heads
        # s-rotation: compile chunk i processes logical slice (g+1+i)%4
        lsl = [(g + 1 + i) % 4 for i in range(4)]
        qT_c = np.concatenate([qT_b[b][:, 512 * l:512 * (l + 1)] for l in lsl],
                              axis=1)
        # arrival chunk ci comes from group-rank r_i = (g-1-ci)%4; its pair-p
        # block maps to Wo/Wg1 input-channel block 2*r_i + p
        ch = [2 * ((g - 1 - ci) % 4) + p for ci in range(4) for p in range(2)]
        woT_c = np.concatenate([WoT[128 * cb:128 * (cb + 1), :] for cb in ch])
        wg1T_c = np.concatenate(
            [Wg1T[0:1024, :]]
            + [Wg1T[1024 + 128 * cb:1024 + 128 * (cb + 1), :] for cb in ch])
        # gather row of (ci, p, head h, dim d):
        #   ci<3: 264*(4b + r_i) + 132p + 66h + d   in rt_og[ci]
        #   ci=3: 264*3        + 132p + 66h + d     in rt_in (own slice)
        def _base(ci, p):
            if ci < 3:
                return 264 * (4 * b + (g - 1 - ci) % 4) + 132 * p
            return 264 * 3 + 132 * p
        _gidx = np.asarray(
            [[_base(kc // 2, kc % 2) + 66 * (j // 64) + (j % 64)
              for j in range(128)] for kc in range(8)], np.int32)
        # denominator rows: j = 2p + h (4 valid); junk rows point at a
        # denominator row too (never zero, keeps 1/x finite)
        _didx = np.asarray(
            [[_base(ci, (j // 2) % 2) + 66 * (j % 2) + 64 if j < 4
              else _base(ci, 0) + 64 for j in range(128)]
             for ci in range(4)], np.int32)
        in_maps.append({
            'qT': np.ascontiguousarray(qT_c),
            'mkT': mkT_b[b],
            'mvT': mvT_b[b],
            'wqT': np.ascontiguousarray((Wq[hs] * scale).T),
            'wkT': np.ascontiguousarray(Wk[hs].T),
            'wvT': np.ascontiguousarray(Wv[hs].T),
            'woT': np.ascontiguousarray(woT_c),
            'wg1T': np.ascontiguousarray(wg1T_c),
            'wg2T': wg2T,
            'qsT': np.ascontiguousarray(q[b].T[:, SSL * g:SSL * (g + 1)]),
            'vones': np.ascontiguousarray(np.tile([1.0, 0.0], 4)[None, :].repeat(128, 0).astype(_bf16)),
            'bc0': bc0,
            'bqv': np.ascontiguousarray((bq[hs] * scale).reshape(2, 128)),
            'bkv': np.ascontiguousarray(bk[hs].reshape(2, 128)),
            'bo2v': np.ascontiguousarray(bo2.reshape(8, 128)),
            'bg1v': np.ascontiguousarray(bg1.reshape(8, 128)),
            'bg2v': bg2v,
            'gidx': _gidx,
            'didx': _didx,
            'sel4': _sel4,
        })
    return in_maps


def _run(inputs, trace=False):
    global _PROG
    from concourse.bass_utils import run_bass_kernel_spmd
    if _PROG is None:
        _PROG = _build_program()
    in_maps = _shard(inputs)
    res = run_bass_kernel_spmd(_PROG, in_maps, list(range(NC)), trace=trace)
    out = np.empty((B, S, DM), np.float32)
    for c in range(NC):
        b, g = c // GS, c % GS
        out[b, SSL * g:SSL * (g + 1), :] = res.results[c]['out_t'].T
    return out, res


def kernel(**inputs) -> np.ndarray:
    out, _ = _run(inputs, trace=False)
    return out
